# revision 6
# baseline (speedup 1.0000x reference)
"""GQA attention kernel for Trainium2, data-parallel over batch on 8 NeuronCores.

Per-core problem (2 of 16 batches): X [1024tok, 1024] -> QKV proj -> RoPE ->
causal GQA attention (8 q heads, 4 kv heads, D=128) -> out proj [1024, 1024].

v4 layout strategy (same math as v1, restructured for PE occupancy):
  - Whole-tensor input DMAs on the two HWDGE rings, ordered by first use
    (X+Wk first) so the PE warmup covers exactly the initial load latency.
  - Phase order K proj -> V proj -> [Q proj interleaved with attention] ->
    out proj.  Interleaving Q chains with attention keeps the PE dense where
    a phase-separated kernel is ACT(exp)-bound during attention, and the
    softmax-normalization tails overlap the next phase's matmuls, so the PE
    never idles >3.4us (which would re-engage the HAM half-clock throttle).
  - Attention is computed transposed (ST[tk,tq]): exp(ST) is directly the
    P.T operand of the PV matmul; colsum via a ones-column matmul.
  - Softmax normalization is per-head and fully off the PE:
    colsum psum -> DVE reciprocal_approx_fast -> GPSIMD partition_broadcast
    -> one DVE multiply that fuses the OT psum->sbuf copy with the scale.
  - Engine assignment is chosen for FIFO-queue cleanliness (strict in-order
    queues mean a slow dependency at the head blocks everything behind it):
      ACT:    exp, psum->sbuf raw copies
      DVE:    rope mul/mul/add (merged [128,1024] tiles), reciprocal, norm
      GPSIMD: causal masks, partition broadcasts (nothing rope-dependent)
      sync DMA ring: X load, rope half-swaps, output stores
      scalar DMA ring: all weight/const loads
  - A global software pipeline (deque) keeps S-matmul emission ~4 j-steps
    ahead of the colsum/PV consumers so the ACT exp + mask latency is hidden,
    and attention consumption lags Q-chain emission by 2 heads so the rope
    chain latency is never on the PE's critical path.
"""

import numpy as np
import ml_dtypes
from collections import deque
from contextlib import ExitStack

import concourse.bass as bass
import concourse.tile as tile
from concourse import bacc, mybir
from concourse.bass_utils import run_bass_kernel_spmd

B, T, HID = 16, 512, 1024
NH, NKV, D = 8, 4, 128
THETA = 10000.0
NCORES = 8
BL = B // NCORES          # local batches per core
TOK = BL * T              # local tokens
P = 128
KT_HID = HID // P         # 8 contraction tiles over hidden
NTQ = T // P              # 4 tk/tq tiles per sequence
GROUPS = NH // NKV        # 2 q heads per kv head
FP32 = mybir.dt.float32
BF16 = mybir.dt.bfloat16
BF = ml_dtypes.bfloat16


def _host_consts():
    inv_freq = 1.0 / (THETA ** (np.arange(0, D, 2, dtype=np.float64) / D))
    freqs = np.outer(np.arange(T, dtype=np.float64), inv_freq)    # [T, 64]
    emb = np.concatenate([freqs, freqs], axis=-1)                 # [T, 128]
    cos = np.cos(emb).T                                           # [128, T]
    sin = np.sin(emb).T
    scale = 1.0 / np.sqrt(D)
    # rotate_half sign folded into sin: out = q*cos + qswap*sin_signed where
    # qswap is q with its partition halves swapped
    sin_signed = np.concatenate([-sin[:D // 2], sin[D // 2:]], axis=0)
    # both local batches are full T-length sequences -> tile along tokens
    cos2 = np.tile(cos, (1, BL))          # [128, TOK]
    sin2 = np.tile(sin_signed, (1, BL))
    # transposed-S diagonal-block multiplicative mask: rows tk, cols tq;
    # valid iff tq >= tk
    mask_t = np.triu(np.ones((P, P), np.float32)).astype(BF)
    return {
        "cos_q": (cos2 * scale).astype(BF),
        "sin_q": (sin2 * scale).astype(BF),
        "cos_k": cos2.astype(BF),
        "sin_k": sin2.astype(BF),
        "mask_t": mask_t,
    }


def _build(nc):
    # hidden arrives pre-transposed from the host: [HID, TOK]
    hid_t = nc.dram_tensor("hidden_t", [HID, TOK], BF16,
                           kind="ExternalInput").ap()
    wq = nc.dram_tensor("Wq", [HID, NH * D], BF16, kind="ExternalInput").ap()
    wk = nc.dram_tensor("Wk", [HID, NKV * D], BF16, kind="ExternalInput").ap()
    wv = nc.dram_tensor("Wv", [HID, NKV * D], BF16, kind="ExternalInput").ap()
    wo = nc.dram_tensor("Wo", [NH * D, HID], BF16, kind="ExternalInput").ap()
    cos_q = nc.dram_tensor("cos_q", [P, TOK], BF16, kind="ExternalInput").ap()
    sin_q = nc.dram_tensor("sin_q", [P, TOK], BF16, kind="ExternalInput").ap()
    cos_k = nc.dram_tensor("cos_k", [P, TOK], BF16, kind="ExternalInput").ap()
    sin_k = nc.dram_tensor("sin_k", [P, TOK], BF16, kind="ExternalInput").ap()
    mask_t = nc.dram_tensor("mask_t", [P, P], BF16, kind="ExternalInput").ap()
    out = nc.dram_tensor("out", [TOK, HID], FP32, kind="ExternalOutput").ap()

    NTOK_T = TOK // P   # 8 token tiles per core
    HALF = D // 2

    with tile.TileContext(nc) as tc, ExitStack() as ctx:
        # ---- pools with cross-phase lifetimes ----
        consts = ctx.enter_context(tc.tile_pool(name="consts", bufs=1))

        cosq_sb = consts.tile([P, TOK], BF16, tag="cq")
        sinq_sb = consts.tile([P, TOK], BF16, tag="sq")
        cosk_sb = consts.tile([P, TOK], BF16, tag="ck")
        sink_sb = consts.tile([P, TOK], BF16, tag="sk")
        maskt_sb = consts.tile([P, P], BF16, tag="maskt")
        ones_bf = consts.tile([P, P], BF16, tag="ones")
        nc.vector.memset(ones_bf, 1.0)
        warm_rhs = consts.tile([P, T], BF16, tag="warm")
        nc.vector.memset(warm_rhs, 0.0)

        qkvpool = ctx.enter_context(tc.tile_pool(name="qkv", bufs=1))
        qt_sb = qkvpool.tile([P, NH, TOK], BF16, tag="qt")     # [d, h, tok]
        kt_sb = qkvpool.tile([P, NKV, TOK], BF16, tag="kt")    # [d, g, tok]
        v_sb = qkvpool.tile([P, NTOK_T, NKV * D], BF16, tag="v")  # [tok,tt,dkv]
        otpool = ctx.enter_context(tc.tile_pool(name="otpool", bufs=1))
        ot_sb = otpool.tile([P, NH, TOK], BF16, tag="ot")      # [d, h, tok]

        wpool = ctx.enter_context(tc.tile_pool(name="wpool", bufs=1))
        wq_sb = wpool.tile([P, KT_HID, NH * D], BF16, tag="wq")
        wk_sb = wpool.tile([P, KT_HID, NKV * D], BF16, tag="wk")
        wv_sb = wpool.tile([P, KT_HID, NKV * D], BF16, tag="wv")
        wo_sb = wpool.tile([P, KT_HID, HID], BF16, tag="wo")
        xt_sb = wpool.tile([P, KT_HID, TOK], BF16, tag="xt")   # [hid, k, tok]

        # ---- input loads: whole-tensor DMAs, ordered by first use ----
        # sync ring: X only (so later rope-swap DMAs aren't queued behind
        # weight transfers); scalar ring: everything else in use order.
        hid_r = hid_t.rearrange("(k p) t -> p k t", p=P)
        wq_r = wq.rearrange("(k p) n -> p k n", p=P)
        wk_r = wk.rearrange("(k p) n -> p k n", p=P)
        wv_r = wv.rearrange("(k p) n -> p k n", p=P)
        wo_r = wo.rearrange("(k p) n -> p k n", p=P)
        for q in range(4):
            k0, k1 = 2 * q, 2 * q + 2
            nc.sync.dma_start(out=xt_sb[:, k0:k1, :], in_=hid_r[:, k0:k1, :])
        nc.scalar.dma_start(out=wk_sb[:, 0:4, :], in_=wk_r[:, 0:4, :])
        nc.scalar.dma_start(out=wk_sb[:, 4:8, :], in_=wk_r[:, 4:8, :])
        nc.scalar.dma_start(out=cosk_sb, in_=cos_k)
        nc.scalar.dma_start(out=sink_sb, in_=sin_k)
        nc.scalar.dma_start(out=wv_sb, in_=wv_r)
        nc.scalar.dma_start(out=wq_sb[:, 0:4, :], in_=wq_r[:, 0:4, :])
        nc.scalar.dma_start(out=wq_sb[:, 4:8, :], in_=wq_r[:, 4:8, :])
        nc.scalar.dma_start(out=cosq_sb, in_=cos_q)
        nc.scalar.dma_start(out=sinq_sb, in_=sin_q)
        nc.scalar.dma_start(out=maskt_sb, in_=mask_t)
        nc.scalar.dma_start(out=wo_sb, in_=wo_r)

        def _rope_finish(raw, out_sl, cos_sb, sin_sb, tmp_pool):
            """out = raw * cos + rotate_half(raw) * sin, on [128, TOK] tiles.

            Partition-half swap runs on the sync DMA ring (compute engines
            cannot shift partitions); all arithmetic on the DVE.
            """
            qswap = tmp_pool.tile([P, TOK], BF16, tag="rope_swap", bufs=2,
                                  name="qswap")
            nc.sync.dma_start(out=qswap[0:HALF], in_=raw[HALF:P])
            nc.sync.dma_start(out=qswap[HALF:P], in_=raw[0:HALF])
            tmp = tmp_pool.tile([P, TOK], BF16, tag="rope_tmp", bufs=2,
                                name="tmp")
            nc.vector.tensor_mul(tmp, qswap, sin_sb)
            nc.vector.tensor_mul(out_sl, raw, cos_sb)
            nc.vector.tensor_add(out_sl, out_sl, tmp)

        # ---- phase A: warmup + K proj + V proj ----
        with ExitStack() as phase1:
            ropet = phase1.enter_context(tc.tile_pool(name="ropetA", bufs=2))
            psA = phase1.enter_context(
                tc.tile_pool(name="psA", bufs=3, space=bass.MemorySpace.PSUM))
            psW = phase1.enter_context(
                tc.tile_pool(name="psW", bufs=1, space=bass.MemorySpace.PSUM))

            # PE warmup: ~9us of dependency-free matmuls so the HAM clock
            # gate releases (1.2 -> 2.4 GHz) while X+Wk are still in flight
            wps = psW.tile([P, T], FP32, tag="warmps")
            for w in range(22):
                nc.tensor.matmul(wps, ones_bf, warm_rhs,
                                 start=True, stop=True, skip_group_check=True)
            for w in range(8):
                nc.tensor.matmul(wps[:, 0:P], ones_bf, warm_rhs[:, 0:P],
                                 start=True, stop=True, skip_group_check=True)

            # KT = Wk.T @ XT + RoPE (rope merged over both batch chunks)
            for g in range(NKV):
                kraw = ropet.tile([P, TOK], BF16, tag="rope_raw", bufs=2,
                                  name="kraw")
                for c in range(BL):
                    ps = psA.tile([P, T], FP32, tag="projps")
                    for k in range(KT_HID):
                        nc.tensor.matmul(
                            ps,
                            wk_sb[:, k, g * P:(g + 1) * P],
                            xt_sb[:, k, c * T:(c + 1) * T],
                            start=(k == 0), stop=(k == KT_HID - 1))
                    nc.scalar.copy(kraw[:, c * T:(c + 1) * T], ps)
                _rope_finish(kraw, kt_sb[:, g, :], cosk_sb, sink_sb, ropet)
            # V natural: [tok, dkv]
            for tt in range(NTOK_T):
                ps = psA.tile([P, T], FP32, tag="projps")
                for k in range(KT_HID):
                    nc.tensor.matmul(
                        ps[:, :NKV * D],
                        xt_sb[:, k, tt * P:(tt + 1) * P],
                        wv_sb[:, k, :],
                        start=(k == 0), stop=(k == KT_HID - 1))
                nc.scalar.copy(v_sb[:, tt, :], ps[:, :NKV * D])

        # ---- phase B: Q proj interleaved with attention ----
        with ExitStack() as phase2:
            ropet = phase2.enter_context(tc.tile_pool(name="ropetB", bufs=2))
            ptpool = phase2.enter_context(tc.tile_pool(name="ptpool", bufs=5))
            stats = phase2.enter_context(tc.tile_pool(name="stats", bufs=3))
            psM = phase2.enter_context(
                tc.tile_pool(name="psM", bufs=1, space=bass.MemorySpace.PSUM))

            pend = deque()
            head_state = {}
            qraw_state = {}

            def emit_qchain(h, c):
                ps = psM.tile([P, T], FP32, tag="qps", bufs=2, name="qps")
                for k in range(KT_HID):
                    nc.tensor.matmul(
                        ps,
                        wq_sb[:, k, h * P:(h + 1) * P],
                        xt_sb[:, k, c * T:(c + 1) * T],
                        start=(k == 0), stop=(k == KT_HID - 1))
                if c == 0:
                    qraw_state[h] = ropet.tile([P, TOK], BF16, tag="rope_raw",
                                               bufs=2, name="qraw")
                qraw = qraw_state[h]
                nc.scalar.copy(qraw[:, c * T:(c + 1) * T], ps)
                if c == BL - 1:
                    _rope_finish(qraw, qt_sb[:, h, :], cosq_sb, sinq_sb,
                                 ropet)
                    del qraw_state[h]

            def emit_S(b, h, j):
                g = h // GROUPS
                lo = j * P
                st_ps = psM.tile([P, T], FP32, tag="sps", bufs=3, name="sps")
                nc.tensor.matmul(
                    st_ps[:, lo:T],
                    kt_sb[:, g, b * T + lo: b * T + lo + P],
                    qt_sb[:, h, b * T + lo: (b + 1) * T],
                    start=True, stop=True)
                # exp -> PT_j, already transposed for the PV matmul
                # (no row-max: logits are O(1) by construction)
                pt = ptpool.tile([P, T], BF16, tag="pt")
                nc.scalar.activation(
                    out=pt[:, lo:T], in_=st_ps[:, lo:T],
                    func=mybir.ActivationFunctionType.Exp,
                    bias=0.0, scale=1.0)
                # causal mask on the diagonal block: multiplicative 0/1 mask
                nc.gpsimd.tensor_mul(pt[:, lo:lo + P], pt[:, lo:lo + P],
                                     maskt_sb)
                pend.append((b, h, j, pt))

            def drain_one():
                b, h, j, pt = pend.popleft()
                g = h // GROUPS
                lo = j * P
                st = head_state.get((b, h))
                if st is None:
                    o_ps_new = psM.tile([P, T], FP32, tag="ops", bufs=2,
                                        name="ops")
                    cs_ps_new = psM.tile([1, T], FP32, tag="cps", bufs=1,
                                         name="cps")
                    st = head_state[(b, h)] = (o_ps_new, cs_ps_new)
                o_ps, cs_ps = st
                # colsum += ones.T @ PT_j ; OT += V_j.T @ PT_j
                nc.tensor.matmul(
                    cs_ps[:, lo:T] if j else cs_ps[:, :],
                    ones_bf[:, 0:1],
                    pt[:, lo:T],
                    start=(j == 0), stop=(j == NTQ - 1),
                    skip_group_check=True)
                nc.tensor.matmul(
                    o_ps[:, lo:T] if j else o_ps[:, :],
                    v_sb[:, b * NTQ + j, g * D:(g + 1) * D],
                    pt[:, lo:T],
                    start=(j == 0), stop=(j == NTQ - 1),
                    skip_group_check=True)
                if j == NTQ - 1:
                    # per-head softmax normalization, entirely off the PE:
                    # 1/colsum -> broadcast over partitions -> fused into the
                    # OT psum->sbuf copy
                    rr = stats.tile([1, T], FP32, tag="rr")
                    nc.vector.reciprocal_approx_fast(rr, cs_ps)
                    rb = stats.tile([P, T], FP32, tag="rb")
                    nc.gpsimd.partition_broadcast(rb, rr)
                    nc.vector.tensor_mul(
                        ot_sb[:, h, b * T:(b + 1) * T], o_ps, rb)
                    del head_state[(b, h)]

            def emit_att(b, h):
                for j in range(NTQ):
                    emit_S(b, h, j)
                    while len(pend) > 3:
                        drain_one()

            # attention lags the Q chains by 2 heads, and within a slot the
            # attention work is emitted FIRST: its DVE/GPSIMD/ACT consumers
            # must sit ahead of the rope ops in each engine's FIFO queue,
            # otherwise the PE's cs/PV matmuls transitively wait on the whole
            # rope latency chain (ACT copy -> swap DMAs -> DVE muls)
            emit_qchain(0, 0)
            emit_qchain(0, 1)
            emit_qchain(1, 0)
            emit_qchain(1, 1)
            for h in range(2, NH):
                emit_att(0, h - 2)
                emit_qchain(h, 0)
                emit_att(1, h - 2)
                emit_qchain(h, 1)
            for h in (NH - 2, NH - 1):
                emit_att(0, h)
                emit_att(1, h)
            while pend:
                drain_one()

        # ---- phase C: output projection ----
        with ExitStack() as phase3:
            opool = phase3.enter_context(tc.tile_pool(name="opool", bufs=3))
            psD = phase3.enter_context(
                tc.tile_pool(name="psD", bufs=4, space=bass.MemorySpace.PSUM))
            NCH = HID // T  # 2 chunks of 512
            for tt in range(NTOK_T):
                o_tile = opool.tile([P, HID], FP32, tag="o")
                # interleave both output chunks k-major: consecutive matmul
                # pairs share the stationary operand OT[:,k,tt-block]
                ps0 = psD.tile([P, T], FP32, tag="dps")
                ps1 = psD.tile([P, T], FP32, tag="dps")
                pss = [ps0, ps1]
                for k in range(KT_HID):
                    for cchunk in range(NCH):
                        nc.tensor.matmul(
                            pss[cchunk],
                            ot_sb[:, k, tt * P:(tt + 1) * P],
                            wo_sb[:, k, cchunk * T:(cchunk + 1) * T],
                            start=(k == 0), stop=(k == KT_HID - 1))
                # alternate engines so the copies run in parallel
                nc.vector.tensor_copy(o_tile[:, 0:T], pss[0])
                nc.scalar.copy(o_tile[:, T:HID], pss[1])
                eng = nc.sync if tt % 2 == 0 else nc.scalar
                eng.dma_start(
                    out=out[tt * P:(tt + 1) * P, :], in_=o_tile)


_COMPILED = None


def _get_compiled():
    global _COMPILED
    if _COMPILED is None:
        nc = bacc.Bacc("TRN2", target_bir_lowering=False, debug=False)
        _build(nc)
        nc.compile()
        _COMPILED = nc
    return _COMPILED


def kernel(hidden_states, Wq, Wk, Wv, Wo, _trace=False, _trace_kwargs=None):
    hs = np.asarray(hidden_states, dtype=np.float32).astype(BF)
    wq = np.ascontiguousarray(np.asarray(Wq, dtype=np.float32).astype(BF))
    wk = np.ascontiguousarray(np.asarray(Wk, dtype=np.float32).astype(BF))
    wv = np.ascontiguousarray(np.asarray(Wv, dtype=np.float32).astype(BF))
    wo = np.ascontiguousarray(np.asarray(Wo, dtype=np.float32).astype(BF))
    consts = _host_consts()
    nc = _get_compiled()
    in_maps = []
    for c in range(NCORES):
        # ship X pre-transposed ([HID, TOK]) so the kernel's lhs/rhs layouts
        # need no on-chip transpose of X at all
        shard_t = np.ascontiguousarray(
            hs[BL * c: BL * (c + 1)].reshape(TOK, HID).T)
        in_maps.append({"hidden_t": shard_t, "Wq": wq, "Wk": wk, "Wv": wv,
                        "Wo": wo, **consts})
    res = run_bass_kernel_spmd(
        nc, in_maps, list(range(NCORES)), trace=_trace,
        **(_trace_kwargs or {}))
    outs = [r["out"].astype(np.float32).reshape(BL, T, HID)
            for r in res.results]
    full = np.concatenate(outs, axis=0)
    if _trace:
        return full, res
    return full


# revision 11
# speedup vs baseline: 1.9773x; 1.9773x over previous
"""GQA attention kernel for Trainium2, data-parallel over batch on 8 NeuronCores.

Per-core problem (2 of 16 batches): X [1024tok, 1024] -> QKV proj -> RoPE ->
causal GQA attention (8 q heads, 4 kv heads, D=128) -> out proj [1024, 1024].

v4 layout strategy (same math as v1, restructured for PE occupancy):
  - Whole-tensor input DMAs on the two HWDGE rings, ordered by first use
    (X+Wk first) so the PE warmup covers exactly the initial load latency.
  - Phase order K proj -> V proj -> [Q proj interleaved with attention] ->
    out proj.  Interleaving Q chains with attention keeps the PE dense where
    a phase-separated kernel is ACT(exp)-bound during attention, and the
    softmax-normalization tails overlap the next phase's matmuls, so the PE
    never idles >3.4us (which would re-engage the HAM half-clock throttle).
  - Attention is computed transposed (ST[tk,tq]): exp(ST) is directly the
    P.T operand of the PV matmul; colsum via a ones-column matmul.
  - Softmax normalization is per-head and fully off the PE:
    colsum psum -> DVE reciprocal_approx_fast -> GPSIMD partition_broadcast
    -> one DVE multiply that fuses the OT psum->sbuf copy with the scale.
  - Engine assignment is chosen for FIFO-queue cleanliness (strict in-order
    queues mean a slow dependency at the head blocks everything behind it):
      ACT:    exp, psum->sbuf raw copies
      DVE:    rope mul/mul/add (merged [128,1024] tiles), reciprocal, norm
      GPSIMD: causal masks, partition broadcasts (nothing rope-dependent)
      sync DMA ring: X load, rope half-swaps, output stores
      scalar DMA ring: all weight/const loads
  - A global software pipeline (deque) keeps S-matmul emission ~4 j-steps
    ahead of the colsum/PV consumers so the ACT exp + mask latency is hidden,
    and attention consumption lags Q-chain emission by 2 heads so the rope
    chain latency is never on the PE's critical path.
"""

import numpy as np
import ml_dtypes
from collections import deque
from contextlib import ExitStack

import concourse.bass as bass
import concourse.tile as tile
from concourse import bacc, mybir
from concourse.bass_utils import run_bass_kernel_spmd

B, T, HID = 16, 512, 1024
NH, NKV, D = 8, 4, 128
THETA = 10000.0
NCORES = 8
BL = B // NCORES          # local batches per core
TOK = BL * T              # local tokens
P = 128
KT_HID = HID // P         # 8 contraction tiles over hidden
NTQ = T // P              # 4 tk/tq tiles per sequence
GROUPS = NH // NKV        # 2 q heads per kv head
FP32 = mybir.dt.float32
BF16 = mybir.dt.bfloat16
BF = ml_dtypes.bfloat16


def _host_consts():
    inv_freq = 1.0 / (THETA ** (np.arange(0, D, 2, dtype=np.float64) / D))
    freqs = np.outer(np.arange(T, dtype=np.float64), inv_freq)    # [T, 64]
    emb = np.concatenate([freqs, freqs], axis=-1)                 # [T, 128]
    cos = np.cos(emb).T                                           # [128, T]
    sin = np.sin(emb).T
    scale = 1.0 / np.sqrt(D)
    # rotate_half sign folded into sin: out = q*cos + qswap*sin_signed where
    # qswap is q with its partition halves swapped
    sin_signed = np.concatenate([-sin[:D // 2], sin[D // 2:]], axis=0)
    # both local batches are full T-length sequences -> tile along tokens
    cos2 = np.tile(cos, (1, BL))          # [128, TOK]
    sin2 = np.tile(sin_signed, (1, BL))
    # transposed-S diagonal-block multiplicative mask: rows tk, cols tq;
    # valid iff tq >= tk
    mask_t = np.triu(np.ones((P, P), np.float32)).astype(BF)
    return {
        "cos_q": (cos2 * scale).astype(BF),
        "sin_q": (sin2 * scale).astype(BF),
        "cos_k": cos2.astype(BF),
        "sin_k": sin2.astype(BF),
        "mask_t": mask_t,
    }


def _build(nc):
    # hidden arrives pre-transposed from the host: [HID, TOK]
    hid_t = nc.dram_tensor("hidden_t", [HID, TOK], BF16,
                           kind="ExternalInput").ap()
    wq = nc.dram_tensor("Wq", [HID, NH * D], BF16, kind="ExternalInput").ap()
    wk = nc.dram_tensor("Wk", [HID, NKV * D], BF16, kind="ExternalInput").ap()
    wv = nc.dram_tensor("Wv", [HID, NKV * D], BF16, kind="ExternalInput").ap()
    wo = nc.dram_tensor("Wo", [NH * D, HID], BF16, kind="ExternalInput").ap()
    cos_q = nc.dram_tensor("cos_q", [P, TOK], BF16, kind="ExternalInput").ap()
    sin_q = nc.dram_tensor("sin_q", [P, TOK], BF16, kind="ExternalInput").ap()
    cos_k = nc.dram_tensor("cos_k", [P, TOK], BF16, kind="ExternalInput").ap()
    sin_k = nc.dram_tensor("sin_k", [P, TOK], BF16, kind="ExternalInput").ap()
    mask_t = nc.dram_tensor("mask_t", [P, P], BF16, kind="ExternalInput").ap()
    out = nc.dram_tensor("out", [TOK, HID], FP32, kind="ExternalOutput").ap()
    # DRAM bounce rows for the softmax-denominator partition broadcast
    rscr = nc.dram_tensor("rscr", [2, T], FP32, kind="Internal").ap()

    NTOK_T = TOK // P   # 8 token tiles per core
    HALF = D // 2

    with tile.TileContext(nc) as tc, ExitStack() as ctx:
        # ---- pools with cross-phase lifetimes ----
        consts = ctx.enter_context(tc.tile_pool(name="consts", bufs=1))

        cosq_sb = consts.tile([P, TOK], BF16, tag="cq")
        sinq_sb = consts.tile([P, TOK], BF16, tag="sq")
        cosk_sb = consts.tile([P, TOK], BF16, tag="ck")
        sink_sb = consts.tile([P, TOK], BF16, tag="sk")
        maskt_sb = consts.tile([P, P], BF16, tag="maskt")
        ones_bf = consts.tile([P, P], BF16, tag="ones")
        nc.vector.memset(ones_bf, 1.0)
        warm_rhs = consts.tile([P, T], BF16, tag="warm")
        nc.vector.memset(warm_rhs, 0.0)

        qkvpool = ctx.enter_context(tc.tile_pool(name="qkv", bufs=1))
        qt_sb = qkvpool.tile([P, NH, TOK], BF16, tag="qt")     # [d, h, tok]
        kt_sb = qkvpool.tile([P, NKV, TOK], BF16, tag="kt")    # [d, g, tok]
        v_sb = qkvpool.tile([P, NTOK_T, NKV * D], BF16, tag="v")  # [tok,tt,dkv]
        otpool = ctx.enter_context(tc.tile_pool(name="otpool", bufs=1))
        ot_sb = otpool.tile([P, NH, TOK], BF16, tag="ot")      # [d, h, tok]

        wpool = ctx.enter_context(tc.tile_pool(name="wpool", bufs=1))
        wq_sb = wpool.tile([P, KT_HID, NH * D], BF16, tag="wq")
        wk_sb = wpool.tile([P, KT_HID, NKV * D], BF16, tag="wk")
        wv_sb = wpool.tile([P, KT_HID, NKV * D], BF16, tag="wv")
        wo_sb = wpool.tile([P, KT_HID, HID], BF16, tag="wo")
        xt_sb = wpool.tile([P, KT_HID, TOK], BF16, tag="xt")   # [hid, k, tok]

        # ---- input loads: whole-tensor DMAs, ordered by first use ----
        # sync ring: X only (so later rope-swap DMAs aren't queued behind
        # weight transfers); scalar ring: everything else in use order.
        hid_r = hid_t.rearrange("(k p) t -> p k t", p=P)
        wq_r = wq.rearrange("(k p) n -> p k n", p=P)
        wk_r = wk.rearrange("(k p) n -> p k n", p=P)
        wv_r = wv.rearrange("(k p) n -> p k n", p=P)
        wo_r = wo.rearrange("(k p) n -> p k n", p=P)
        for q in range(4):
            k0, k1 = 2 * q, 2 * q + 2
            nc.sync.dma_start(out=xt_sb[:, k0:k1, :], in_=hid_r[:, k0:k1, :])
        nc.scalar.dma_start(out=wk_sb[:, 0:4, :], in_=wk_r[:, 0:4, :])
        nc.scalar.dma_start(out=wk_sb[:, 4:8, :], in_=wk_r[:, 4:8, :])
        nc.scalar.dma_start(out=cosk_sb, in_=cos_k)
        nc.scalar.dma_start(out=sink_sb, in_=sin_k)
        nc.scalar.dma_start(out=wv_sb, in_=wv_r)
        nc.scalar.dma_start(out=wq_sb[:, 0:4, :], in_=wq_r[:, 0:4, :])
        nc.scalar.dma_start(out=wq_sb[:, 4:8, :], in_=wq_r[:, 4:8, :])
        nc.scalar.dma_start(out=cosq_sb, in_=cos_q)
        nc.scalar.dma_start(out=sinq_sb, in_=sin_q)
        nc.scalar.dma_start(out=maskt_sb, in_=mask_t)
        nc.scalar.dma_start(out=wo_sb, in_=wo_r)

        def _rope_finish(raw, out_sl, cos_sb, sin_sb, tmp_pool):
            """out = raw * cos + rotate_half(raw) * sin, on [128, TOK] tiles.

            Partition-half swap runs on the sync DMA ring (compute engines
            cannot shift partitions); all arithmetic on the DVE.
            """
            qswap = tmp_pool.tile([P, TOK], BF16, tag="rope_swap", bufs=2,
                                  name="qswap")
            nc.sync.dma_start(out=qswap[0:HALF], in_=raw[HALF:P])
            nc.sync.dma_start(out=qswap[HALF:P], in_=raw[0:HALF])
            tmp = tmp_pool.tile([P, TOK], BF16, tag="rope_tmp", bufs=2,
                                name="tmp")
            # tmp-mul on GPSIMD (its only op type all kernel -> no Q7
            # library reloads); the rest on DVE
            nc.gpsimd.tensor_mul(tmp, qswap, sin_sb)
            nc.vector.tensor_mul(out_sl, raw, cos_sb)
            nc.vector.tensor_add(out_sl, out_sl, tmp)

        # ---- phase A: warmup + K proj + V proj ----
        with ExitStack() as phase1:
            ropet = phase1.enter_context(tc.tile_pool(name="ropetA", bufs=2))
            psA = phase1.enter_context(
                tc.tile_pool(name="psA", bufs=3, space=bass.MemorySpace.PSUM))
            psW = phase1.enter_context(
                tc.tile_pool(name="psW", bufs=1, space=bass.MemorySpace.PSUM))

            # PE warmup: ~9us of dependency-free matmuls so the HAM clock
            # gate releases (1.2 -> 2.4 GHz) while X+Wk are still in flight
            wps = psW.tile([P, T], FP32, tag="warmps")
            for w in range(22):
                nc.tensor.matmul(wps, ones_bf, warm_rhs,
                                 start=True, stop=True, skip_group_check=True)
            for w in range(8):
                nc.tensor.matmul(wps[:, 0:P], ones_bf, warm_rhs[:, 0:P],
                                 start=True, stop=True, skip_group_check=True)

            # KT = Wk.T @ XT + RoPE (rope merged over both batch chunks)
            for g in range(NKV):
                kraw = ropet.tile([P, TOK], BF16, tag="rope_raw", bufs=2,
                                  name="kraw")
                for c in range(BL):
                    ps = psA.tile([P, T], FP32, tag="projps")
                    for k in range(KT_HID):
                        nc.tensor.matmul(
                            ps,
                            wk_sb[:, k, g * P:(g + 1) * P],
                            xt_sb[:, k, c * T:(c + 1) * T],
                            start=(k == 0), stop=(k == KT_HID - 1))
                    nc.scalar.copy(kraw[:, c * T:(c + 1) * T], ps)
                _rope_finish(kraw, kt_sb[:, g, :], cosk_sb, sink_sb, ropet)
            # V natural: [tok, dkv]
            for tt in range(NTOK_T):
                ps = psA.tile([P, T], FP32, tag="projps")
                for k in range(KT_HID):
                    nc.tensor.matmul(
                        ps[:, :NKV * D],
                        xt_sb[:, k, tt * P:(tt + 1) * P],
                        wv_sb[:, k, :],
                        start=(k == 0), stop=(k == KT_HID - 1))
                nc.scalar.copy(v_sb[:, tt, :], ps[:, :NKV * D])

        # ---- phase B: Q proj interleaved with attention ----
        with ExitStack() as phase2:
            ropet = phase2.enter_context(tc.tile_pool(name="ropetB", bufs=2))
            ptpool = phase2.enter_context(tc.tile_pool(name="ptpool", bufs=5))
            stats = phase2.enter_context(tc.tile_pool(name="stats", bufs=3))
            psM = phase2.enter_context(
                tc.tile_pool(name="psM", bufs=1, space=bass.MemorySpace.PSUM))

            pend = deque()
            head_state = {}
            qraw_state = {}

            def emit_qchain(h, c):
                ps = psM.tile([P, T], FP32, tag="qps", bufs=2, name="qps")
                for k in range(KT_HID):
                    nc.tensor.matmul(
                        ps,
                        wq_sb[:, k, h * P:(h + 1) * P],
                        xt_sb[:, k, c * T:(c + 1) * T],
                        start=(k == 0), stop=(k == KT_HID - 1))
                if c == 0:
                    qraw_state[h] = ropet.tile([P, TOK], BF16, tag="rope_raw",
                                               bufs=2, name="qraw")
                qraw = qraw_state[h]
                nc.scalar.copy(qraw[:, c * T:(c + 1) * T], ps)
                if c == BL - 1:
                    _rope_finish(qraw, qt_sb[:, h, :], cosq_sb, sinq_sb,
                                 ropet)
                    del qraw_state[h]

            def emit_S(b, h, j):
                g = h // GROUPS
                lo = j * P
                st_ps = psM.tile([P, T], FP32, tag="sps", bufs=3, name="sps")
                nc.tensor.matmul(
                    st_ps[:, lo:T],
                    kt_sb[:, g, b * T + lo: b * T + lo + P],
                    qt_sb[:, h, b * T + lo: (b + 1) * T],
                    start=True, stop=True)
                # exp -> PT_j, already transposed for the PV matmul
                # (no row-max: logits are O(1) by construction)
                pt = ptpool.tile([P, T], BF16, tag="pt")
                nc.scalar.activation(
                    out=pt[:, lo:T], in_=st_ps[:, lo:T],
                    func=mybir.ActivationFunctionType.Exp,
                    bias=0.0, scale=1.0)
                # causal mask on the diagonal block: multiplicative 0/1 mask
                nc.vector.tensor_mul(pt[:, lo:lo + P], pt[:, lo:lo + P],
                                     maskt_sb)
                pend.append((b, h, j, pt))

            def drain_one():
                b, h, j, pt = pend.popleft()
                g = h // GROUPS
                lo = j * P
                st = head_state.get((b, h))
                if st is None:
                    o_ps_new = psM.tile([P, T], FP32, tag="ops", bufs=2,
                                        name="ops")
                    cs_ps_new = psM.tile([1, T], FP32, tag="cps", bufs=1,
                                         name="cps")
                    st = head_state[(b, h)] = (o_ps_new, cs_ps_new)
                o_ps, cs_ps = st
                # colsum += ones.T @ PT_j ; OT += V_j.T @ PT_j
                nc.tensor.matmul(
                    cs_ps[:, lo:T] if j else cs_ps[:, :],
                    ones_bf[:, 0:1],
                    pt[:, lo:T],
                    start=(j == 0), stop=(j == NTQ - 1),
                    skip_group_check=True)
                nc.tensor.matmul(
                    o_ps[:, lo:T] if j else o_ps[:, :],
                    v_sb[:, b * NTQ + j, g * D:(g + 1) * D],
                    pt[:, lo:T],
                    start=(j == 0), stop=(j == NTQ - 1),
                    skip_group_check=True)
                if j == NTQ - 1:
                    # per-head softmax normalization, entirely off the PE:
                    # 1/colsum -> broadcast over partitions -> fused into the
                    # OT psum->sbuf copy
                    rr = stats.tile([1, T], FP32, tag="rr")
                    nc.vector.reciprocal_approx_fast(rr, cs_ps)
                    # partition-broadcast via a DRAM bounce + 0-stride DMA
                    # read on the otherwise-idle sync ring (GPSIMD's
                    # partition_broadcast forces a Q7 library swap against
                    # tensor_tensor ops; SBUF DMA sources reject 0-stride)
                    slot = (b * NH + h) % 2
                    nc.sync.dma_start(out=rscr[slot:slot + 1, :], in_=rr)
                    rb = stats.tile([P, T], FP32, tag="rb")
                    nc.sync.dma_start(
                        out=rb, in_=rscr[slot:slot + 1, :].to_broadcast((P, T)))
                    nc.vector.tensor_mul(
                        ot_sb[:, h, b * T:(b + 1) * T], o_ps, rb)
                    del head_state[(b, h)]

            def emit_att(b, h):
                for j in range(NTQ):
                    emit_S(b, h, j)
                    while len(pend) > 3:
                        drain_one()

            # attention lags the Q chains by 2 heads, and within a slot the
            # attention work is emitted FIRST: its DVE/GPSIMD/ACT consumers
            # must sit ahead of the rope ops in each engine's FIFO queue,
            # otherwise the PE's cs/PV matmuls transitively wait on the whole
            # rope latency chain (ACT copy -> swap DMAs -> DVE muls)
            emit_qchain(0, 0)
            emit_qchain(0, 1)
            emit_qchain(1, 0)
            emit_qchain(1, 1)
            for h in range(2, NH):
                emit_att(0, h - 2)
                emit_qchain(h, 0)
                emit_att(1, h - 2)
                emit_qchain(h, 1)
            for h in (NH - 2, NH - 1):
                emit_att(0, h)
                emit_att(1, h)
            while pend:
                drain_one()

        # ---- phase C: output projection ----
        with ExitStack() as phase3:
            opool = phase3.enter_context(tc.tile_pool(name="opool", bufs=3))
            psD = phase3.enter_context(
                tc.tile_pool(name="psD", bufs=4, space=bass.MemorySpace.PSUM))
            NCH = HID // T  # 2 chunks of 512
            for tt in range(NTOK_T):
                o_tile = opool.tile([P, HID], FP32, tag="o")
                # interleave both output chunks k-major: consecutive matmul
                # pairs share the stationary operand OT[:,k,tt-block]
                ps0 = psD.tile([P, T], FP32, tag="dps")
                ps1 = psD.tile([P, T], FP32, tag="dps")
                pss = [ps0, ps1]
                for k in range(KT_HID):
                    for cchunk in range(NCH):
                        nc.tensor.matmul(
                            pss[cchunk],
                            ot_sb[:, k, tt * P:(tt + 1) * P],
                            wo_sb[:, k, cchunk * T:(cchunk + 1) * T],
                            start=(k == 0), stop=(k == KT_HID - 1))
                # alternate engines so the copies run in parallel
                nc.vector.tensor_copy(o_tile[:, 0:T], pss[0])
                nc.scalar.copy(o_tile[:, T:HID], pss[1])
                eng = nc.sync if tt % 2 == 0 else nc.scalar
                eng.dma_start(
                    out=out[tt * P:(tt + 1) * P, :], in_=o_tile)


_COMPILED = None


def _get_compiled():
    global _COMPILED
    if _COMPILED is None:
        nc = bacc.Bacc("TRN2", target_bir_lowering=False, debug=False)
        _build(nc)
        nc.compile()
        _COMPILED = nc
    return _COMPILED


def kernel(hidden_states, Wq, Wk, Wv, Wo, _trace=False, _trace_kwargs=None):
    hs = np.asarray(hidden_states, dtype=np.float32).astype(BF)
    wq = np.ascontiguousarray(np.asarray(Wq, dtype=np.float32).astype(BF))
    wk = np.ascontiguousarray(np.asarray(Wk, dtype=np.float32).astype(BF))
    wv = np.ascontiguousarray(np.asarray(Wv, dtype=np.float32).astype(BF))
    wo = np.ascontiguousarray(np.asarray(Wo, dtype=np.float32).astype(BF))
    consts = _host_consts()
    nc = _get_compiled()
    in_maps = []
    for c in range(NCORES):
        # ship X pre-transposed ([HID, TOK]) so the kernel's lhs/rhs layouts
        # need no on-chip transpose of X at all
        shard_t = np.ascontiguousarray(
            hs[BL * c: BL * (c + 1)].reshape(TOK, HID).T)
        in_maps.append({"hidden_t": shard_t, "Wq": wq, "Wk": wk, "Wv": wv,
                        "Wo": wo, **consts})
    res = run_bass_kernel_spmd(
        nc, in_maps, list(range(NCORES)), trace=_trace,
        **(_trace_kwargs or {}))
    outs = [r["out"].astype(np.float32).reshape(BL, T, HID)
            for r in res.results]
    full = np.concatenate(outs, axis=0)
    if _trace:
        return full, res
    return full


# revision 14
# speedup vs baseline: 2.0635x; 1.0436x over previous
"""GQA attention kernel for Trainium2, data-parallel over batch on 8 NeuronCores.

Per-core problem (2 of 16 batches): X [1024tok, 1024] -> QKV proj -> RoPE ->
causal GQA attention (8 q heads, 4 kv heads, D=128) -> out proj [1024, 1024].

v4 layout strategy (same math as v1, restructured for PE occupancy):
  - Whole-tensor input DMAs on the two HWDGE rings, ordered by first use
    (X+Wk first) so the PE warmup covers exactly the initial load latency.
  - Phase order K proj -> V proj -> [Q proj interleaved with attention] ->
    out proj.  Interleaving Q chains with attention keeps the PE dense where
    a phase-separated kernel is ACT(exp)-bound during attention, and the
    softmax-normalization tails overlap the next phase's matmuls, so the PE
    never idles >3.4us (which would re-engage the HAM half-clock throttle).
  - Attention is computed transposed (ST[tk,tq]): exp(ST) is directly the
    P.T operand of the PV matmul; colsum via a ones-column matmul.
  - Softmax normalization is per-head and fully off the PE:
    colsum psum -> DVE reciprocal_approx_fast -> GPSIMD partition_broadcast
    -> one DVE multiply that fuses the OT psum->sbuf copy with the scale.
  - Engine assignment is chosen for FIFO-queue cleanliness (strict in-order
    queues mean a slow dependency at the head blocks everything behind it):
      ACT:    exp, psum->sbuf raw copies
      DVE:    rope mul/mul/add (merged [128,1024] tiles), reciprocal, norm
      GPSIMD: causal masks, partition broadcasts (nothing rope-dependent)
      sync DMA ring: X load, rope half-swaps, output stores
      scalar DMA ring: all weight/const loads
  - A global software pipeline (deque) keeps S-matmul emission ~4 j-steps
    ahead of the colsum/PV consumers so the ACT exp + mask latency is hidden,
    and attention consumption lags Q-chain emission by 2 heads so the rope
    chain latency is never on the PE's critical path.
"""

import numpy as np
import ml_dtypes
from collections import deque
from contextlib import ExitStack

import concourse.bass as bass
import concourse.tile as tile
from concourse import bacc, mybir
from concourse.bass_utils import run_bass_kernel_spmd

B, T, HID = 16, 512, 1024
NH, NKV, D = 8, 4, 128
THETA = 10000.0
NCORES = 8
BL = B // NCORES          # local batches per core
TOK = BL * T              # local tokens
P = 128
KT_HID = HID // P         # 8 contraction tiles over hidden
NTQ = T // P              # 4 tk/tq tiles per sequence
GROUPS = NH // NKV        # 2 q heads per kv head
FP32 = mybir.dt.float32
BF16 = mybir.dt.bfloat16
BF = ml_dtypes.bfloat16


def _host_consts():
    inv_freq = 1.0 / (THETA ** (np.arange(0, D, 2, dtype=np.float64) / D))
    freqs = np.outer(np.arange(T, dtype=np.float64), inv_freq)    # [T, 64]
    emb = np.concatenate([freqs, freqs], axis=-1)                 # [T, 128]
    cos = np.cos(emb).T                                           # [128, T]
    sin = np.sin(emb).T
    scale = 1.0 / np.sqrt(D)
    # rotate_half sign folded into sin: out = q*cos + qswap*sin_signed where
    # qswap is q with its partition halves swapped
    sin_signed = np.concatenate([-sin[:D // 2], sin[D // 2:]], axis=0)
    # both local batches are full T-length sequences -> tile along tokens
    cos2 = np.tile(cos, (1, BL))          # [128, TOK]
    sin2 = np.tile(sin_signed, (1, BL))
    # transposed-S diagonal-block multiplicative mask: rows tk, cols tq;
    # valid iff tq >= tk
    mask_t = np.triu(np.ones((P, P), np.float32)).astype(BF)
    return {
        "cos_q": (cos2 * scale).astype(BF),
        "sin_q": (sin2 * scale).astype(BF),
        "cos_k": cos2.astype(BF),
        "sin_k": sin2.astype(BF),
        "mask_t": mask_t,
    }


def _build(nc):
    # hidden arrives pre-transposed from the host: [HID, TOK]
    hid_t = nc.dram_tensor("hidden_t", [HID, TOK], BF16,
                           kind="ExternalInput").ap()
    wq = nc.dram_tensor("Wq", [HID, NH * D], BF16, kind="ExternalInput").ap()
    wk = nc.dram_tensor("Wk", [HID, NKV * D], BF16, kind="ExternalInput").ap()
    wv = nc.dram_tensor("Wv", [HID, NKV * D], BF16, kind="ExternalInput").ap()
    wo = nc.dram_tensor("Wo", [NH * D, HID], BF16, kind="ExternalInput").ap()
    cos_q = nc.dram_tensor("cos_q", [P, TOK], BF16, kind="ExternalInput").ap()
    sin_q = nc.dram_tensor("sin_q", [P, TOK], BF16, kind="ExternalInput").ap()
    cos_k = nc.dram_tensor("cos_k", [P, TOK], BF16, kind="ExternalInput").ap()
    sin_k = nc.dram_tensor("sin_k", [P, TOK], BF16, kind="ExternalInput").ap()
    mask_t = nc.dram_tensor("mask_t", [P, P], BF16, kind="ExternalInput").ap()
    out = nc.dram_tensor("out", [TOK, HID], FP32, kind="ExternalOutput").ap()
    # DRAM bounce rows for the softmax-denominator partition broadcast
    rscr = nc.dram_tensor("rscr", [2, T], FP32, kind="Internal").ap()

    NTOK_T = TOK // P   # 8 token tiles per core
    HALF = D // 2

    with tile.TileContext(nc) as tc, ExitStack() as ctx:
        # ---- pools with cross-phase lifetimes ----
        consts = ctx.enter_context(tc.tile_pool(name="consts", bufs=1))

        cosq_sb = consts.tile([P, TOK], BF16, tag="cq")
        sinq_sb = consts.tile([P, TOK], BF16, tag="sq")
        cosk_sb = consts.tile([P, TOK], BF16, tag="ck")
        sink_sb = consts.tile([P, TOK], BF16, tag="sk")
        maskt_sb = consts.tile([P, P], BF16, tag="maskt")
        ones_bf = consts.tile([P, P], BF16, tag="ones")
        nc.vector.memset(ones_bf, 1.0)
        warm_rhs = consts.tile([P, T], BF16, tag="warm")
        nc.vector.memset(warm_rhs, 0.0)

        qkvpool = ctx.enter_context(tc.tile_pool(name="qkv", bufs=1))
        qt_sb = qkvpool.tile([P, NH, TOK], BF16, tag="qt")     # [d, h, tok]
        kt_sb = qkvpool.tile([P, NKV, TOK], BF16, tag="kt")    # [d, g, tok]
        v_sb = qkvpool.tile([P, NTOK_T, NKV * D], BF16, tag="v")  # [tok,tt,dkv]
        otpool = ctx.enter_context(tc.tile_pool(name="otpool", bufs=1))
        ot_sb = otpool.tile([P, NH, TOK], BF16, tag="ot")      # [d, h, tok]

        wpool = ctx.enter_context(tc.tile_pool(name="wpool", bufs=1))
        wq_sb = wpool.tile([P, KT_HID, NH * D], BF16, tag="wq")
        wk_sb = wpool.tile([P, KT_HID, NKV * D], BF16, tag="wk")
        wv_sb = wpool.tile([P, KT_HID, NKV * D], BF16, tag="wv")
        wo_sb = wpool.tile([P, KT_HID, HID], BF16, tag="wo")
        xt_sb = wpool.tile([P, KT_HID, TOK], BF16, tag="xt")   # [hid, k, tok]

        # ---- input loads: whole-tensor DMAs, ordered by first use ----
        # sync ring: X only (so later rope-swap DMAs aren't queued behind
        # weight transfers); scalar ring: everything else in use order.
        hid_r = hid_t.rearrange("(k p) t -> p k t", p=P)
        wq_r = wq.rearrange("(k p) n -> p k n", p=P)
        wk_r = wk.rearrange("(k p) n -> p k n", p=P)
        wv_r = wv.rearrange("(k p) n -> p k n", p=P)
        wo_r = wo.rearrange("(k p) n -> p k n", p=P)
        nc.sync.dma_start(out=xt_sb[:, 0:2, :], in_=hid_r[:, 0:2, :])
        nc.sync.dma_start(out=xt_sb[:, 2:4, :], in_=hid_r[:, 2:4, :])
        nc.sync.dma_start(out=xt_sb[:, 4:6, :], in_=hid_r[:, 4:6, :])
        nc.scalar.dma_start(out=wk_sb[:, 0:4, :], in_=wk_r[:, 0:4, :])
        nc.scalar.dma_start(out=wk_sb[:, 4:8, :], in_=wk_r[:, 4:8, :])
        nc.scalar.dma_start(out=xt_sb[:, 6:8, :], in_=hid_r[:, 6:8, :])
        nc.scalar.dma_start(out=cosk_sb, in_=cos_k)
        nc.scalar.dma_start(out=sink_sb, in_=sin_k)
        nc.scalar.dma_start(out=wv_sb, in_=wv_r)
        nc.scalar.dma_start(out=wq_sb[:, 0:4, :], in_=wq_r[:, 0:4, :])
        nc.scalar.dma_start(out=wq_sb[:, 4:8, :], in_=wq_r[:, 4:8, :])
        nc.scalar.dma_start(out=cosq_sb, in_=cos_q)
        nc.scalar.dma_start(out=sinq_sb, in_=sin_q)
        nc.scalar.dma_start(out=maskt_sb, in_=mask_t)
        nc.scalar.dma_start(out=wo_sb, in_=wo_r)

        def _rope_finish(raw, out_sl, cos_sb, sin_sb, tmp_pool):
            """out = raw * cos + rotate_half(raw) * sin, on [128, TOK] tiles.

            Partition-half swap runs on the sync DMA ring (compute engines
            cannot shift partitions); all arithmetic on the DVE.
            """
            qswap = tmp_pool.tile([P, TOK], BF16, tag="rope_swap", bufs=2,
                                  name="qswap")
            nc.sync.dma_start(out=qswap[0:HALF], in_=raw[HALF:P])
            nc.sync.dma_start(out=qswap[HALF:P], in_=raw[0:HALF])
            tmp = tmp_pool.tile([P, TOK], BF16, tag="rope_tmp", bufs=2,
                                name="tmp")
            # tmp-mul on GPSIMD (its only op type all kernel -> no Q7
            # library reloads); the rest on DVE
            nc.gpsimd.tensor_mul(tmp, qswap, sin_sb)
            nc.vector.tensor_mul(out_sl, raw, cos_sb)
            nc.vector.tensor_add(out_sl, out_sl, tmp)

        # ---- phase A: warmup + K proj + V proj ----
        with ExitStack() as phase1:
            ropet = phase1.enter_context(tc.tile_pool(name="ropetA", bufs=2))
            psA = phase1.enter_context(
                tc.tile_pool(name="psA", bufs=3, space=bass.MemorySpace.PSUM))
            psW = phase1.enter_context(
                tc.tile_pool(name="psW", bufs=1, space=bass.MemorySpace.PSUM))

            # PE warmup: ~9us of dependency-free matmuls so the HAM clock
            # gate releases (1.2 -> 2.4 GHz) while X+Wk are still in flight
            wps = psW.tile([P, T], FP32, tag="warmps")
            for w in range(22):
                nc.tensor.matmul(wps, ones_bf, warm_rhs,
                                 start=True, stop=True, skip_group_check=True)
            for w in range(8):
                nc.tensor.matmul(wps[:, 0:P], ones_bf, warm_rhs[:, 0:P],
                                 start=True, stop=True, skip_group_check=True)

            # KT = Wk.T @ XT + RoPE (rope merged over both batch chunks)
            for g in range(NKV):
                kraw = ropet.tile([P, TOK], BF16, tag="rope_raw", bufs=2,
                                  name="kraw")
                for c in range(BL):
                    ps = psA.tile([P, T], FP32, tag="projps")
                    for k in range(KT_HID):
                        nc.tensor.matmul(
                            ps,
                            wk_sb[:, k, g * P:(g + 1) * P],
                            xt_sb[:, k, c * T:(c + 1) * T],
                            start=(k == 0), stop=(k == KT_HID - 1))
                    nc.scalar.copy(kraw[:, c * T:(c + 1) * T], ps)
                _rope_finish(kraw, kt_sb[:, g, :], cosk_sb, sink_sb, ropet)
            # V natural: [tok, dkv]
            for tt in range(NTOK_T):
                ps = psA.tile([P, T], FP32, tag="projps")
                for k in range(KT_HID):
                    nc.tensor.matmul(
                        ps[:, :NKV * D],
                        xt_sb[:, k, tt * P:(tt + 1) * P],
                        wv_sb[:, k, :],
                        start=(k == 0), stop=(k == KT_HID - 1))
                nc.scalar.copy(v_sb[:, tt, :], ps[:, :NKV * D])

        # ---- phase B: Q proj interleaved with attention ----
        with ExitStack() as phase2:
            ropet = phase2.enter_context(tc.tile_pool(name="ropetB", bufs=2))
            ptpool = phase2.enter_context(tc.tile_pool(name="ptpool", bufs=5))
            stats = phase2.enter_context(tc.tile_pool(name="stats", bufs=3))
            psM = phase2.enter_context(
                tc.tile_pool(name="psM", bufs=1, space=bass.MemorySpace.PSUM))

            pend = deque()
            head_state = {}
            qraw_state = {}

            def emit_qchain(h, c):
                ps = psM.tile([P, T], FP32, tag="qps", bufs=2, name="qps")
                for k in range(KT_HID):
                    nc.tensor.matmul(
                        ps,
                        wq_sb[:, k, h * P:(h + 1) * P],
                        xt_sb[:, k, c * T:(c + 1) * T],
                        start=(k == 0), stop=(k == KT_HID - 1))
                if c == 0:
                    qraw_state[h] = ropet.tile([P, TOK], BF16, tag="rope_raw",
                                               bufs=2, name="qraw")
                qraw = qraw_state[h]
                nc.scalar.copy(qraw[:, c * T:(c + 1) * T], ps)
                if c == BL - 1:
                    _rope_finish(qraw, qt_sb[:, h, :], cosq_sb, sinq_sb,
                                 ropet)
                    del qraw_state[h]

            def emit_S(b, h, j):
                g = h // GROUPS
                lo = j * P
                st_ps = psM.tile([P, T], FP32, tag="sps", bufs=3, name="sps")
                nc.tensor.matmul(
                    st_ps[:, lo:T],
                    kt_sb[:, g, b * T + lo: b * T + lo + P],
                    qt_sb[:, h, b * T + lo: (b + 1) * T],
                    start=True, stop=True)
                # exp -> PT_j, already transposed for the PV matmul
                # (no row-max: logits are O(1) by construction)
                pt = ptpool.tile([P, T], BF16, tag="pt")
                nc.scalar.activation(
                    out=pt[:, lo:T], in_=st_ps[:, lo:T],
                    func=mybir.ActivationFunctionType.Exp,
                    bias=0.0, scale=1.0)
                # causal mask on the diagonal block: multiplicative 0/1 mask
                nc.vector.tensor_mul(pt[:, lo:lo + P], pt[:, lo:lo + P],
                                     maskt_sb)
                pend.append((b, h, j, pt))

            def drain_one():
                b, h, j, pt = pend.popleft()
                g = h // GROUPS
                lo = j * P
                st = head_state.get((b, h))
                if st is None:
                    o_ps_new = psM.tile([P, T], FP32, tag="ops", bufs=2,
                                        name="ops")
                    cs_ps_new = psM.tile([1, T], FP32, tag="cps", bufs=1,
                                         name="cps")
                    st = head_state[(b, h)] = (o_ps_new, cs_ps_new)
                o_ps, cs_ps = st
                # colsum += ones.T @ PT_j ; OT += V_j.T @ PT_j
                nc.tensor.matmul(
                    cs_ps[:, lo:T] if j else cs_ps[:, :],
                    ones_bf[:, 0:1],
                    pt[:, lo:T],
                    start=(j == 0), stop=(j == NTQ - 1),
                    skip_group_check=True)
                nc.tensor.matmul(
                    o_ps[:, lo:T] if j else o_ps[:, :],
                    v_sb[:, b * NTQ + j, g * D:(g + 1) * D],
                    pt[:, lo:T],
                    start=(j == 0), stop=(j == NTQ - 1),
                    skip_group_check=True)
                if j == NTQ - 1:
                    # per-head softmax normalization, entirely off the PE:
                    # 1/colsum -> broadcast over partitions -> fused into the
                    # OT psum->sbuf copy
                    rr = stats.tile([1, T], FP32, tag="rr")
                    nc.vector.reciprocal_approx_fast(rr, cs_ps)
                    # partition-broadcast via a DRAM bounce + 0-stride DMA
                    # read on the otherwise-idle sync ring (GPSIMD's
                    # partition_broadcast forces a Q7 library swap against
                    # tensor_tensor ops; SBUF DMA sources reject 0-stride)
                    slot = (b * NH + h) % 2
                    nc.sync.dma_start(out=rscr[slot:slot + 1, :], in_=rr)
                    rb = stats.tile([P, T], FP32, tag="rb")
                    nc.sync.dma_start(
                        out=rb, in_=rscr[slot:slot + 1, :].to_broadcast((P, T)))
                    nc.vector.tensor_mul(
                        ot_sb[:, h, b * T:(b + 1) * T], o_ps, rb)
                    del head_state[(b, h)]

            def emit_att(b, h):
                for j in range(NTQ):
                    emit_S(b, h, j)
                    while len(pend) > 3:
                        drain_one()

            def emit_oproj(tt, cchunk):
                # one out-projection chunk: out[tt-block, chunk] as its own
                # k-chain, sharing the qps psum rotation with the (finished)
                # Q chains so it can interleave with the attention tail
                ps = psM.tile([P, T], FP32, tag="qps", bufs=2, name="opps")
                for k in range(KT_HID):
                    nc.tensor.matmul(
                        ps,
                        ot_sb[:, k, tt * P:(tt + 1) * P],
                        wo_sb[:, k, cchunk * T:(cchunk + 1) * T],
                        start=(k == 0), stop=(k == KT_HID - 1))
                o_tile = stats.tile([P, T], FP32, tag="oout", bufs=3,
                                    name="o_tile")
                if (2 * tt + cchunk) % 2 == 0:
                    nc.vector.tensor_copy(o_tile, ps)
                else:
                    nc.scalar.copy(o_tile, ps)
                eng = nc.sync if (2 * tt + cchunk) % 2 == 0 else nc.scalar
                eng.dma_start(
                    out=out[tt * P:(tt + 1) * P,
                            cchunk * T:(cchunk + 1) * T],
                    in_=o_tile)

            # attention lags the Q chains by 2 heads, and within a slot the
            # attention work is emitted FIRST: its DVE/GPSIMD/ACT consumers
            # must sit ahead of the rope ops in each engine's FIFO queue,
            # otherwise the PE's cs/PV matmuls transitively wait on the whole
            # rope latency chain (ACT copy -> swap DMAs -> DVE muls)
            emit_qchain(0, 0)
            emit_qchain(0, 1)
            emit_qchain(1, 0)
            emit_qchain(1, 1)
            for h in range(2, NH):
                emit_att(0, h - 2)
                emit_qchain(h, 0)
                emit_att(1, h - 2)
                emit_qchain(h, 1)
            # tail: interleave out-projection chunks (batch-0 token tiles
            # first) with the remaining ACT-bound attention so the PE stays
            # dense and the HAM clock gate never re-throttles
            emit_att(0, NH - 2)
            emit_att(0, NH - 1)
            emit_att(1, NH - 2)          # drains + norms (0, NH-1) inside
            emit_oproj(0, 0)
            emit_oproj(0, 1)
            emit_att(1, NH - 1)
            emit_oproj(1, 0)
            emit_oproj(1, 1)
            while pend:
                drain_one()
            for tt in range(2, NTOK_T):
                emit_oproj(tt, 0)
                emit_oproj(tt, 1)


_COMPILED = None


def _get_compiled():
    global _COMPILED
    if _COMPILED is None:
        nc = bacc.Bacc("TRN2", target_bir_lowering=False, debug=False)
        _build(nc)
        nc.compile()
        _COMPILED = nc
    return _COMPILED


def kernel(hidden_states, Wq, Wk, Wv, Wo, _trace=False, _trace_kwargs=None):
    hs = np.asarray(hidden_states, dtype=np.float32).astype(BF)
    wq = np.ascontiguousarray(np.asarray(Wq, dtype=np.float32).astype(BF))
    wk = np.ascontiguousarray(np.asarray(Wk, dtype=np.float32).astype(BF))
    wv = np.ascontiguousarray(np.asarray(Wv, dtype=np.float32).astype(BF))
    wo = np.ascontiguousarray(np.asarray(Wo, dtype=np.float32).astype(BF))
    consts = _host_consts()
    nc = _get_compiled()
    in_maps = []
    for c in range(NCORES):
        # ship X pre-transposed ([HID, TOK]) so the kernel's lhs/rhs layouts
        # need no on-chip transpose of X at all
        shard_t = np.ascontiguousarray(
            hs[BL * c: BL * (c + 1)].reshape(TOK, HID).T)
        in_maps.append({"hidden_t": shard_t, "Wq": wq, "Wk": wk, "Wv": wv,
                        "Wo": wo, **consts})
    res = run_bass_kernel_spmd(
        nc, in_maps, list(range(NCORES)), trace=_trace,
        **(_trace_kwargs or {}))
    outs = [r["out"].astype(np.float32).reshape(BL, T, HID)
            for r in res.results]
    full = np.concatenate(outs, axis=0)
    if _trace:
        return full, res
    return full
